# revision 30
# baseline (speedup 1.0000x reference)
"""MAB (multihead attention block) Trainium2 kernel, v4.

Sharding: 8 cores = 4 batches x 2 query-halves. Each core computes, for its
batch b and query half s (1024 queries), the full 8-head attention block:
    q = Q @ Wq.T + bq ; k = V @ Wk.T + bk ; v = V @ Wv.T   (bv folded out)
    S = q k^T / sqrt(512); masked softmax over keys; O = q + A @ v + bv
    out = O + relu(O @ Wo.T + bo)

Precision plan (rel-err budget 2e-2):
  - the residual q path and the output projection stay bf16 (their error
    is not averaged down by the key contraction)
  - V/Wk/Wv/Wq-for-logits and the softmax weights (es) are fp8e4: their
    quantization error averages out over the 2048-key softmax
  - fp8 enables DoubleRow matmuls (two contraction tiles per pass) for the
    v/k/q8 projections, the attention logits, and the attention numerator,
    roughly halving PE time vs bf16
  - PSUM accumulation is always f32

Logits DoubleRow needs the head_dim=64 contraction split as 32 partitions
x 2 free-slots, so k and the fp8 copy of q live in a split-feature layout
kt8/qt8[P, pr, j, n]: partitions 0-31 = even head feats 32j..32j+31,
partitions 32-63 = odd head feats 32j..32j+31 (weight columns permuted on
the host to make the projection land directly in this layout).

Schedule: one flat region, paced by the ACT engine streaming one exp per
key-chunk step (the ~132us floor). All projection work is chopped into
<=0.5us quarter-passes interleaved into attention-step slack. The key mask
is folded into the exp bias column; the softmax denominator rides as a
ones-column in the v tile so the numerator matmul accumulates it for free.
"""

import math
import os

import numpy as np

import concourse.bass as bass
import concourse.tile as tile
from concourse import bacc, mybir

F32 = mybir.dt.float32
BF16 = mybir.dt.bfloat16
FP8 = mybir.dt.float8e4
DR = mybir.MatmulPerfMode.DoubleRow

DIM = 512
NQ = 1024  # queries per core
NK = 2048  # keys per core
P = 128
FCH = DIM // P  # 4 feature chunks (= head pairs)
KD = DIM // P  # 4 contraction chunks (2 DoubleRow pairs)
TCH = NK // P  # 16 token/key chunks
QCH = NQ // 512  # 2 query chunks of 512
SCALE = 1.0 / math.sqrt(DIM)
MASK_NEG = -30000.0

# v_sb per-token-chunk column layout: 4 even-head blocks of 65 (v[64] | one),
# then 4 odd-head blocks of 128 (one | zeros[63] | v[64]), then 12 pad cols
# so the chunk stride is a multiple of 16 bytes (DoubleRow lhsT AP rule).
VW = 4 * 65 + 4 * 128 + 12  # 784
VUSED = 772
EVEN_OFF = [65 * i for i in range(4)]
ODD_OFF = [260 + 128 * i for i in range(4)]

INPUT_SPECS = {
    "QT": ((DIM, NQ), BF16),
    "QT8": ((DIM, NQ), FP8),
    "VT": ((DIM, NK), FP8),
    "WqT": ((DIM, DIM), BF16),
    "Wq8": ((DIM, DIM), FP8),   # columns permuted for the split-feature layout
    "Wk8": ((DIM, DIM), FP8),   # columns permuted for the split-feature layout
    "WvTp": ((DIM, VUSED), FP8),
    "WoT": ((DIM, DIM), BF16),
    "bq": ((DIM,), F32),
    "bqp": ((DIM,), F32),       # permuted to match Wq8 column order
    "bkp": ((DIM,), F32),       # permuted to match Wk8 column order
    "bv": ((DIM,), F32),
    "bo": ((DIM,), F32),
    "mlog": ((NK,), F32),
}


def emit(ctx, tc, io):
    """Emit the kernel. io: dict name -> DRAM AP (inputs + 'outT')."""
    nc = tc.nc
    AF = mybir.ActivationFunctionType
    OP = mybir.AluOpType

    consts = ctx.enter_context(tc.tile_pool(name="consts", bufs=1))
    bigs = ctx.enter_context(tc.tile_pool(name="bigs", bufs=1))

    # warm the ACT exp table early so the ~2.7us table load overlaps DMA
    warm = consts.tile([1, 1], F32)
    nc.vector.memset(warm, 0.0)
    nc.scalar.activation(warm, warm, AF.Exp)

    # all-ones stationary operand for the PE denominator-broadcast
    ones_sb = consts.tile([P, 64], BF16)
    nc.vector.memset(ones_sb, 1.0)

    # ---- weights / inputs ------------------------------------------------
    # Few, large DMAs (SP dispatch is serial), strictly in first-use order:
    # k8 quarters (VT+wk8+bkp), q8 quarters (wq8+bqp), first exp (mlog, QT8),
    # v passes (wvp), then the rest.
    vt_src = io["VT"].rearrange("(kd p) t -> p kd t", p=P)
    vtin = bigs.tile([P, KD, NK], FP8)
    nc.sync.dma_start(vtin[:, :, 0:512], vt_src[:, :, 0:512])
    wk8_sb = bigs.tile([P, KD, DIM], FP8)
    nc.sync.dma_start(wk8_sb, io["Wk8"].rearrange("(kd p) f -> p kd f", p=P))
    bkp_sb = consts.tile([P, 8], F32)
    nc.sync.dma_start(bkp_sb[0:64, :], io["bkp"].rearrange("(g p) -> p g", p=64))
    nc.sync.dma_start(vtin[:, :, 512:1024], vt_src[:, :, 512:1024])
    wq8_sb = bigs.tile([P, KD, DIM], FP8)
    nc.sync.dma_start(wq8_sb, io["Wq8"].rearrange("(kd p) f -> p kd f", p=P))
    bqp_sb = consts.tile([P, 8], F32)
    nc.sync.dma_start(bqp_sb[0:64, :], io["bqp"].rearrange("(g p) -> p g", p=64))
    mlog_sb = consts.tile([P, TCH], F32)
    nc.sync.dma_start(mlog_sb, io["mlog"].rearrange("(c p) -> p c", p=P))
    qtin8 = bigs.tile([P, KD, NQ], FP8)
    nc.sync.dma_start(qtin8, io["QT8"].rearrange("(kd p) t -> p kd t", p=P))
    wvp_sb = bigs.tile([P, KD, VUSED], FP8)
    nc.sync.dma_start(wvp_sb, io["WvTp"].rearrange("(kd p) f -> p kd f", p=P))
    nc.sync.dma_start(vtin[:, :, 1024:1536], vt_src[:, :, 1024:1536])
    bv_sb = consts.tile([P, FCH], F32)
    nc.sync.dma_start(bv_sb, io["bv"].rearrange("(c p) -> p c", p=P))
    nc.sync.dma_start(vtin[:, :, 1536:2048], vt_src[:, :, 1536:2048])
    bo_sb = consts.tile([P, FCH], F32)
    nc.sync.dma_start(bo_sb, io["bo"].rearrange("(c p) -> p c", p=P))
    bq_sb = consts.tile([P, FCH], F32)
    nc.sync.dma_start(bq_sb, io["bq"].rearrange("(c p) -> p c", p=P))
    wq_sb = bigs.tile([P, KD, DIM], BF16)
    nc.sync.dma_start(wq_sb, io["WqT"].rearrange("(kd p) f -> p kd f", p=P))
    qtin = bigs.tile([P, KD, NQ], BF16)
    nc.sync.dma_start(qtin, io["QT"].rearrange("(kd p) t -> p kd t", p=P))
    wo_sb = bigs.tile([P, KD, DIM], BF16)
    nc.sync.dma_start(wo_sb, io["WoT"].rearrange("(kd p) f -> p kd f", p=P))

    # ---- persistent results ----------------------------------------------
    v_sb = bigs.tile([P, TCH, VW], FP8)
    kt8 = bigs.tile([P, FCH, 2, NK], FP8)   # rows 0-63 used (split-feature)
    qt8 = bigs.tile([P, FCH, 2, NQ], FP8)   # rows 0-63 used (split-feature)
    qt_sb = bigs.tile([P, FCH, NQ], BF16)   # residual-path q (feature-major)
    ot_sb = bigs.tile([P, FCH, NQ], BF16)

    # ---- pools -----------------------------------------------------------
    ps_s = ctx.enter_context(tc.tile_pool(name="ps_s", bufs=3, space="PSUM"))
    ps_n = ctx.enter_context(tc.tile_pool(name="ps_n", bufs=2, space="PSUM"))
    att = ctx.enter_context(tc.tile_pool(name="att", bufs=3))
    sm = ctx.enter_context(tc.tile_pool(name="sm", bufs=2))
    dr = ctx.enter_context(tc.tile_pool(name="dr", bufs=2, space="DRAM"))

    # ---- projection quarter-passes (transient users of the ps_s ring) ----
    def v_pass(t):
        """Project v for key chunk t: token-major [128 tokens, 772]."""
        ps_v = ps_s.tile([P, VUSED], F32, tag="s", padded_shape=[P, 1024],
                         name="ps_v")
        for g in range(2):  # DoubleRow kd pairs
            lhsT = vtin[:, 2 * g:2 * g + 2, t * P:(t + 1) * P]
            nc.tensor.matmul(
                ps_v[:, 0:512], lhsT, wvp_sb[:, 2 * g:2 * g + 2, 0:512],
                start=(g == 0), stop=(g == 1), perf_mode=DR,
            )
            nc.tensor.matmul(
                ps_v[:, 512:VUSED], lhsT, wvp_sb[:, 2 * g:2 * g + 2, 512:VUSED],
                start=(g == 0), stop=(g == 1), perf_mode=DR,
            )
        nc.vector.tensor_copy(v_sb[:, t, 0:VUSED], ps_v)

    def ones_pair(c):
        """Set the denominator ones-columns for key chunks 2c, 2c+1."""
        ev = v_sb[:, 2 * c:2 * c + 2, 0:260].rearrange(
            "p t (e c) -> p t e c", c=65)[:, :, :, 64]
        nc.vector.memset(ev, 1.0)
        od = v_sb[:, 2 * c:2 * c + 2, 260:772].rearrange(
            "p t (o c) -> p t o c", c=128)[:, :, :, 0]
        nc.vector.memset(od, 1.0)

    def k8_quarter(pr, j, n):
        """Project k split-feature group (pr,j) for key cols n*512.. (fp8 DR)."""
        g = 2 * pr + j
        ps = ps_s.tile([P, 512], F32, tag="s", padded_shape=[P, 1024], name="ps_k8")
        for gg in range(2):
            nc.tensor.matmul(
                ps[0:64, :], wk8_sb[:, 2 * gg:2 * gg + 2, g * 64:(g + 1) * 64],
                vtin[:, 2 * gg:2 * gg + 2, n * 512:(n + 1) * 512],
                start=(gg == 0), stop=(gg == 1), perf_mode=DR,
            )
        nc.vector.tensor_scalar_add(
            kt8[0:64, pr, j, n * 512:(n + 1) * 512], ps[0:64, :],
            bkp_sb[0:64, g:g + 1],
        )

    def q8_quarter(pr, j, n):
        """Project logits-q split-feature group (pr,j) for q cols n*512.."""
        g = 2 * pr + j
        ps = ps_s.tile([P, 512], F32, tag="s", padded_shape=[P, 1024], name="ps_q8")
        for gg in range(2):
            nc.tensor.matmul(
                ps[0:64, :], wq8_sb[:, 2 * gg:2 * gg + 2, g * 64:(g + 1) * 64],
                qtin8[:, 2 * gg:2 * gg + 2, n * 512:(n + 1) * 512],
                start=(gg == 0), stop=(gg == 1), perf_mode=DR,
            )
        nc.vector.tensor_scalar_add(
            qt8[0:64, pr, j, n * 512:(n + 1) * 512], ps[0:64, :],
            bqp_sb[0:64, g:g + 1],
        )

    def qbf_quarter(fc, n):
        """Residual-path q (bf16, feature-major) for q cols n*512.."""
        ps = ps_s.tile([P, 512], F32, tag="s", padded_shape=[P, 1024], name="ps_q")
        for kd in range(KD):
            nc.tensor.matmul(
                ps, wq_sb[:, kd, fc * P:(fc + 1) * P],
                qtin[:, kd, n * 512:(n + 1) * 512],
                start=(kd == 0), stop=(kd == KD - 1),
            )
        nc.vector.tensor_scalar_add(
            qt_sb[:, fc, n * 512:(n + 1) * 512], ps, bq_sb[:, fc:fc + 1]
        )

    out_dst = io["outT"].rearrange("(fc p) q -> p fc q", p=P)

    def out_finish(ups, qc, ofc, use_act=False):
        qsl = slice(qc * 512, (qc + 1) * 512)
        r1 = sm.tile([P, 512], BF16, tag="r1")
        if use_act:  # end of kernel: exp stream is done, ACT is free
            nc.scalar.activation(r1, ups, AF.Relu, bias=bo_sb[:, ofc:ofc + 1])
        else:
            nc.vector.tensor_scalar(
                r1, ups, bo_sb[:, ofc:ofc + 1], 0.0, op0=OP.add, op1=OP.max
            )
        fin = sm.tile([P, 512], F32, tag="fin")
        nc.vector.tensor_tensor(fin, r1, ot_sb[:, ofc, qsl], op=OP.add)
        nc.sync.dma_start(out_dst[:, ofc, qsl], fin)

    def out_quarter(qc, ofc, use_act=False):
        """Full output-projection block for (qc, ofc) via the s ring."""
        qsl = slice(qc * 512, (qc + 1) * 512)
        ups = ps_s.tile([P, 512], F32, tag="s", padded_shape=[P, 1024], name="ups")
        for ifc in range(FCH):
            nc.tensor.matmul(
                ups, wo_sb[:, ifc, ofc * P:(ofc + 1) * P], ot_sb[:, ifc, qsl],
                start=(ifc == 0), stop=(ifc == FCH - 1),
            )
        out_finish(ups, qc, ofc, use_act=use_act)

    # ---- attention --------------------------------------------------------
    state = {}

    def att_begin(pr, qc):
        state["num0"] = ps_n.tile([P, 512], F32, tag="num", name="num0")
        state["num1"] = ps_n.tile([P, 512], F32, tag="num", name="num1")

    def att_step(pr, qc, kc):
        s_ps = ps_s.tile([P, 1024], F32, tag="s", name="s_ps")
        for hh in range(2):
            nc.tensor.matmul(
                s_ps[:, hh * 512:(hh + 1) * 512],
                kt8[32 * hh:32 * hh + 32, pr, :, kc * P:(kc + 1) * P],
                qt8[32 * hh:32 * hh + 32, pr, :, qc * 512:(qc + 1) * 512],
                start=True, stop=True, perf_mode=DR,
                tile_position=(32 * hh, 0),
            )
        if kc % 2 == 0:
            state["es2"] = att.tile([P, 2, 1024], FP8, tag="es", name="es2")
        es2 = state["es2"]
        nc.scalar.activation(es2[:, kc % 2, :], s_ps, AF.Exp,
                             bias=mlog_sb[:, kc:kc + 1], scale=SCALE)

    def num_pair(pr, qc, c, num0, num1, es2):
        """fp8 DoubleRow numerator over the (2c, 2c+1) chunk pair."""
        off = EVEN_OFF[pr]
        nc.tensor.matmul(
            num0[0:65, :], v_sb[:, 2 * c:2 * c + 2, off:off + 65],
            es2[:, :, 0:512],
            start=(c == 0), stop=(c == TCH // 2 - 1), perf_mode=DR,
        )
        off = ODD_OFF[pr]
        nc.tensor.matmul(
            num1, v_sb[:, 2 * c:2 * c + 2, off:off + 128],
            es2[:, :, 512:1024],
            start=(c == 0), stop=(c == TCH // 2 - 1), perf_mode=DR,
        )

    def att_tail(pr, qc, fast=False):
        num0, num1 = state["num0"], state["num1"]
        qsl = slice(qc * 512, (qc + 1) * 512)
        # drain PSUM fast (frees the 2-deep num ring for the next head pair):
        # reciprocals of the two denominator rows + numerator copies to SBUF
        if not fast:
            rec0 = sm.tile([65, 512], F32, tag="rec0")
            nc.vector.reciprocal(rec0[64:65, :], num0[64:65, :])
            rec1 = sm.tile([P, 512], F32, tag="rec1")
            nc.vector.reciprocal(rec1[0:1, :], num1[0:1, :])
        nab = sm.tile([P, 512], F32, tag="nab")
        nc.vector.tensor_copy(nab[0:64, :], num0[0:64, :])
        nc.vector.tensor_copy(nab[64:128, :], num1[64:128, :])
        if fast:
            # final tail: PE is idle, broadcast the reciprocal rows with a
            # K=1 bf16 ones-matmul (lower latency than the DMA round trip)
            rb0 = sm.tile([65, 512], BF16, tag="rb0")
            rb1 = sm.tile([P, 512], BF16, tag="rb1")
            with nc.allow_low_precision(reason="softmax 1/den scale, 2e-2 budget"):
                nc.vector.reciprocal(rb0[64:65, :], num0[64:65, :])
                nc.vector.reciprocal(rb1[0:1, :], num1[0:1, :])
            bca_ps = ps_s.tile([P, 512], F32, tag="s", padded_shape=[P, 1024],
                               name="bca_ps")
            nc.tensor.matmul(
                bca_ps[0:64, :], ones_sb[64:65, :], rb0[64:65, :],
                start=True, stop=True, tile_position=(64, 0),
            )
            nc.tensor.matmul(
                bca_ps[64:128, :], ones_sb[0:1, :], rb1[0:1, :],
                start=True, stop=True, tile_position=(0, 64),
            )
            bca = bca_ps
        else:
            # mid-kernel: DMA round-trip broadcast, fully off the PE path
            dr2 = dr.tile([2, 512], F32, tag="drec")
            nc.sync.dma_start(dr2[0:1, :], rec0[64:65, :])
            nc.sync.dma_start(dr2[1:2, :], rec1[0:1, :])
            bca = sm.tile([P, 512], F32, tag="bca")
            nc.sync.dma_start(bca[0:64, :], dr2[0:1, :].broadcast_to([64, 512]))
            nc.sync.dma_start(bca[64:128, :], dr2[1:2, :].broadcast_to([64, 512]))
        t1 = sm.tile([P, 512], BF16, tag="t1")
        nc.vector.tensor_tensor(t1, nab, bca, op=OP.mult)
        nc.vector.scalar_tensor_tensor(
            ot_sb[:, pr, qsl], t1, bv_sb[:, pr:pr + 1], qt_sb[:, pr, qsl],
            op0=OP.add, op1=OP.add,
        )

    # ---- fused schedule ---------------------------------------------------
    # prologue: just what attention step 0 needs -- kt8 fc0 cols 0-1023 and
    # qt8 fc0 qc0. The v chunks stream inside block (0,0), two per step,
    # always one pair ahead of the numerator matmuls that consume them.
    for n in range(2):
        for j in range(2):
            k8_quarter(0, j, n)
    q8_quarter(0, 0, 0)
    q8_quarter(0, 1, 0)

    K8 = k8_quarter
    Q8 = q8_quarter
    QB = qbf_quarter
    inserts = {
        # block pr carries: its own cols-1024+ k8 spill (kc1-4), the next
        # pair's cols 0-1023 k8 (kc5-8), qc1 prep (kc9-11), and the next
        # pair's q8/qbf (kc12-15). The ramp block (0,0) skips the spill rule.
        (0, 0, 2): lambda: K8(0, 0, 2),
        (0, 0, 3): lambda: K8(0, 1, 2),
        (0, 0, 4): lambda: K8(0, 0, 3),
        (0, 0, 5): lambda: K8(0, 1, 3),
        (0, 0, 6): lambda: K8(1, 0, 0),
        (0, 0, 7): lambda: K8(1, 1, 0),
        (0, 0, 8): lambda: K8(1, 0, 1),
        (0, 0, 9): lambda: K8(1, 1, 1),
        (0, 0, 10): lambda: Q8(1, 0, 0),
        (0, 0, 11): lambda: Q8(1, 1, 0),
        (0, 0, 12): lambda: QB(1, 0),
        (0, 0, 13): lambda: QB(0, 0),
        (0, 1, 1): lambda: K8(1, 0, 2),
        (0, 1, 2): lambda: K8(1, 1, 2),
        (0, 1, 3): lambda: K8(1, 0, 3),
        (0, 1, 4): lambda: K8(1, 1, 3),
        (0, 1, 5): lambda: K8(2, 0, 0),
        (0, 1, 6): lambda: K8(2, 1, 0),
        (0, 1, 7): lambda: K8(2, 0, 1),
        (0, 1, 8): lambda: K8(2, 1, 1),
        (0, 1, 9): lambda: Q8(0, 0, 1),
        (0, 1, 10): lambda: Q8(0, 1, 1),
        (0, 1, 11): lambda: QB(0, 1),
        (0, 1, 12): lambda: Q8(2, 0, 0),
        (0, 1, 13): lambda: Q8(2, 1, 0),
        (0, 1, 14): lambda: QB(2, 0),
        (0, 2, 1): lambda: K8(2, 0, 2),
        (0, 2, 2): lambda: K8(2, 1, 2),
        (0, 2, 3): lambda: K8(2, 0, 3),
        (0, 2, 4): lambda: K8(2, 1, 3),
        (0, 2, 5): lambda: K8(3, 0, 0),
        (0, 2, 6): lambda: K8(3, 1, 0),
        (0, 2, 7): lambda: K8(3, 0, 1),
        (0, 2, 8): lambda: K8(3, 1, 1),
        (0, 2, 9): lambda: Q8(1, 0, 1),
        (0, 2, 10): lambda: Q8(1, 1, 1),
        (0, 2, 11): lambda: QB(1, 1),
        (0, 2, 12): lambda: Q8(3, 0, 0),
        (0, 2, 13): lambda: Q8(3, 1, 0),
        (0, 2, 14): lambda: QB(3, 0),
        (0, 3, 1): lambda: K8(3, 0, 2),
        (0, 3, 2): lambda: K8(3, 1, 2),
        (0, 3, 3): lambda: K8(3, 0, 3),
        (0, 3, 4): lambda: K8(3, 1, 3),
        (0, 3, 5): lambda: Q8(2, 0, 1),
        (0, 3, 6): lambda: Q8(2, 1, 1),
        (0, 3, 7): lambda: QB(2, 1),
        (0, 3, 8): lambda: Q8(3, 0, 1),
        (0, 3, 9): lambda: Q8(3, 1, 1),
        (0, 3, 10): lambda: QB(3, 1),
        (1, 0, 3): lambda: out_quarter(0, 0),
        (1, 0, 7): lambda: out_quarter(0, 1),
        (1, 0, 11): lambda: out_quarter(0, 2),
        (1, 0, 15): lambda: out_quarter(0, 3),
    }

    deferred = None  # (pr, qc, num0, num1, es2) of the previous block's last pair
    for qc in range(QCH):
        for pr in range(FCH):
            att_begin(pr, qc)
            num0, num1 = state["num0"], state["num1"]
            for kc in range(TCH):
                att_step(pr, qc, kc)
                if kc == 0 and deferred is not None:
                    # previous block: last numerator pair, then its softmax
                    # tail -- after this block's first logits so the exp
                    # stream never waits on them
                    dpr, dqc, dn0, dn1, des2 = deferred
                    num_pair(dpr, dqc, TCH // 2 - 1, dn0, dn1, des2)
                    state["num0"], state["num1"] = dn0, dn1
                    att_tail(dpr, dqc)
                    state["num0"], state["num1"] = num0, num1
                    deferred = None
                if kc % 2 == 1:
                    if kc == TCH - 1:
                        deferred = (pr, qc, num0, num1, state["es2"])
                    else:
                        num_pair(pr, qc, kc // 2, num0, num1, state["es2"])
                if qc == 0 and pr == 0:
                    if kc == 0:
                        v_pass(0)
                        v_pass(1)
                        ones_pair(0)
                    elif kc < 15:
                        v_pass(kc + 1)
                        if kc % 2 == 0:
                            ones_pair(kc // 2)
                ins = inserts.get((qc, pr, kc))
                if ins is not None:
                    ins()
    dpr, dqc, dn0, dn1, des2 = deferred
    num_pair(dpr, dqc, TCH // 2 - 1, dn0, dn1, des2)
    state["num0"], state["num1"] = dn0, dn1
    att_tail(dpr, dqc, fast=True)
    for ofc in range(FCH):
        out_quarter(1, ofc, use_act=True)


def _perm_cols(W):
    """Column permutation for the split-feature layout: group g=2*pr+j holds
    [head(2pr) feats 32j..32j+31, head(2pr+1) feats 32j..32j+31]."""
    cols = []
    for pr in range(4):
        for j in range(2):
            cols.extend(range((2 * pr) * 64 + 32 * j, (2 * pr) * 64 + 32 * j + 32))
            cols.extend(range((2 * pr + 1) * 64 + 32 * j, (2 * pr + 1) * 64 + 32 * j + 32))
    return np.asarray(cols)


def make_core_inputs(Q, V, mask, Wq, bq, Wk, bk, Wv, bv, Wo, bo, core):
    import ml_dtypes

    BF = ml_dtypes.bfloat16
    F8 = ml_dtypes.float8_e4m3fn
    b, s = divmod(core, 2)
    f32 = np.float32
    QT = np.ascontiguousarray(Q[b, s * NQ:(s + 1) * NQ, :].T)
    VT = np.ascontiguousarray(V[b].T).astype(F8)
    WvT = np.ascontiguousarray(Wv.T, dtype=f32)
    WvTp = np.zeros((DIM, VUSED), dtype=f32)
    for i in range(4):  # even heads 2i
        WvTp[:, EVEN_OFF[i]:EVEN_OFF[i] + 64] = WvT[:, (2 * i) * 64:(2 * i + 1) * 64]
    for i in range(4):  # odd heads 2i+1
        WvTp[:, ODD_OFF[i] + 64:ODD_OFF[i] + 128] = WvT[:, (2 * i + 1) * 64:(2 * i + 2) * 64]
    perm = _perm_cols(None)
    WqT_f = np.ascontiguousarray(Wq.T, dtype=f32)
    WkT_f = np.ascontiguousarray(Wk.T, dtype=f32)
    mlog = np.where(np.asarray(mask[b], bool), 0.0, MASK_NEG).astype(f32)
    return {
        "QT": QT.astype(BF),
        "QT8": QT.astype(F8),
        "VT": VT,
        "WqT": WqT_f.astype(BF),
        "Wq8": np.ascontiguousarray(WqT_f[:, perm]).astype(F8),
        "Wk8": np.ascontiguousarray(WkT_f[:, perm]).astype(F8),
        "WvTp": WvTp.astype(F8),
        "WoT": np.ascontiguousarray(Wo.T).astype(BF),
        "bq": np.asarray(bq, dtype=f32),
        "bqp": np.asarray(bq, dtype=f32)[perm],
        "bkp": np.asarray(bk, dtype=f32)[perm],
        "bv": np.asarray(bv, dtype=f32),
        "bo": np.asarray(bo, dtype=f32),
        "mlog": mlog,
    }


_CACHE = {}


def build_program():
    if "nc" in _CACHE:
        return _CACHE["nc"]
    from contextlib import ExitStack

    nc = bacc.Bacc("TRN2", target_bir_lowering=False, debug=False)
    io = {}
    for name, (shape, dt) in INPUT_SPECS.items():
        io[name] = nc.dram_tensor(name, list(shape), dt, kind="ExternalInput").ap()
    io["outT"] = nc.dram_tensor("outT", [DIM, NQ], F32, kind="ExternalOutput").ap()
    with tile.TileContext(nc) as tc:
        with ExitStack() as ctx:
            emit(ctx, tc, io)
    nc.compile()
    _CACHE["nc"] = nc
    return nc


def kernel(Q, V, mask, Wq, bq, Wk, bk, Wv, bv, Wo, bo):
    from concourse.bass_utils import run_bass_kernel_spmd

    nc = build_program()
    args = (Q, V, mask, Wq, bq, Wk, bk, Wv, bv, Wo, bo)
    in_maps = [make_core_inputs(*args, core=c) for c in range(8)]
    res = run_bass_kernel_spmd(
        nc, in_maps, core_ids=list(range(8)),
        trace=bool(int(os.environ.get("KTRACE", "0"))),
    )
    _CACHE["last_result"] = res
    B = 4
    out = np.empty((B, 2 * NQ, DIM), np.float32)
    for c in range(8):
        b, s = divmod(c, 2)
        out[b, s * NQ:(s + 1) * NQ, :] = res.results[c]["outT"].T
    return out
